# revision 7
# baseline (speedup 1.0000x reference)
"""Trainium2 Bass kernel: Performer (linear) attention + in/out projections.

Problem nn_LinearPerformerAttention_6717328851263:
  x:(4,4096,1024) f32, w_qkv:(1024,3072), proj_matrix:(16,64,256),
  w_out:(1024,1024), b_out:(1024,)

  qkv = x @ w_qkv ; split q,k,v ; per (b,h): q_proj=elu1(q@P_h), k_proj=elu1(k@P_h)
  kv = k_proj^T v ; k_sum = sum_n k_proj ; attn = (q_proj @ kv) / (q_proj@k_sum)
  out = attn @ w_out + b_out

Sharding over 8 cores: core c -> (batch b=c//2, head-group g=c%2: 8 of 16 heads).
Each core computes partial y_c = attn(b, heads_g) @ w_out[512g:512g+512, :].
Host gather: out[b] = y_(b,0) + y_(b,1) + b_out.

v4 (from v3 at ~437us):
  * qT/kT computed with fp8(e4m3) DoubleRow matmuls (2 k-slabs per pass,
    0.5 cyc/row): x and wq/wk are shipped in fp8, weights pre-scaled x32 on
    the host (w std 1/32 -> ~1.0) and de-scaled at the PSUM evict (ACT Copy
    scale=1/32).  Host-simmed end-to-end rel err 7.9e-3 (gate 2e-2); v stays
    bf16 (fp8 v measured 3.2e-2 - fails).
  * kv state accumulates directly in 4 persistent PSUM banks across all 8
    token groups (start at g=0, stop at g=NG-1) - kills the per-group
    PSUM->SBUF fold (DVE -2.6us/group).  Pass-A PSUM = mm(2)+kp(2)+kv(4).
  * elu1 = min(exp(c),1) + relu(c): Exp always on ACT; second op pattern-
    split between ACT (Relu) and DVE (max+1); third op (min/[+]) offloaded
    to the otherwise-idle GPSIMD (SBUF-only operands) to unload ACT/DVE.
  * pass B re-ordered for a gapless PE stream (p-state ramp: any idle gap
    costs ~3us of half-speed): per group emit [attn pair-hp x4 interleaved
    with qproj(g+1) heads 0-3], then [y(g-1) chunks x8 interleaved with
    qproj heads 4-7].  attn+denom share one 2-bank PSUM tile; qproj psum is
    a single [128,1024] ring; y chunks are [128,512] o-split ring-2.
    Pass-B PSUM = attn(2x2)+qproj(2)+y(2x1).
  * all 8 heads of group 0's qproj warm-started in the pass-A epilogue,
    interleaved with the kv fixup (transpose + kvS/ksr bf16 packing).
"""

import numpy as np
from contextlib import ExitStack

import ml_dtypes
import concourse.bass as bass
import concourse.bacc as bacc
import concourse.tile as tile
from concourse import mybir
from concourse.bass_utils import run_bass_kernel_spmd
from concourse.masks import make_identity

FP32 = mybir.dt.float32
BF16 = mybir.dt.bfloat16
FP8 = mybir.dt.float8e4
AL = mybir.AluOpType
AF = mybir.ActivationFunctionType
DR = mybir.MatmulPerfMode.DoubleRow

B, SEQ, D = 4, 4096, 1024
H, HD, F = 16, 64, 256
HPC = 8            # heads per core
DH = HPC * HD      # 512 head-space dims per core
P = 128
NCORES = 8
W8SCALE = 32.0     # host pre-scale on fp8 wq/wk (w std 1/32 -> ~1)

# elu1 engine assignment patterns (tuning knobs).
# second op: 'S' = ACT Relu, 'V' = DVE max(c,0)+1  (third op - the SBUF
# bf16 min/stt - always runs on DVE: walrus rejects non-Add/Mult ALU ops
# on the Pool engine, so no GPSIMD offload)
SECOND_A = "VVVVVVVSVVVVVVVS"   # pass A: 16 kproj tiles/group
SECOND_B = "SVSSVSSV"           # pass B: 8 qproj tiles/group


def _emit(tc, n, xT, xT8, wq8, wk8, wv, proj, wout, y):
    nc = tc.nc
    NG = n // 512       # token groups
    TPG = 4             # 128-token tiles per group

    def emit_elu(c_ps, out, pool, second, tag, w=512):
        """elu1 = min(exp(c),1) + relu(c), c in PSUM fp32 -> bf16 `out`."""
        e = pool.tile([P, w], BF16, tag=f"{tag}E", name=f"{tag}E")
        r = pool.tile([P, w], BF16, tag=f"{tag}R", name=f"{tag}R")
        nc.scalar.activation(e, c_ps, AF.Exp)
        if second == "S":
            nc.scalar.activation(r, c_ps, AF.Relu)
            # min(e,1) + r
            nc.vector.scalar_tensor_tensor(
                out, in0=e, scalar=1.0, in1=r, op0=AL.min, op1=AL.add)
        else:
            # r = relu(c)+1 ; out = min(e, r)  (== min(e,1)+relu(c))
            nc.vector.tensor_scalar(r, c_ps, 0.0, 1.0, op0=AL.max, op1=AL.add)
            nc.vector.tensor_tensor(out=out, in0=e, in1=r, op=AL.min)

    ctx = ExitStack()
    with ctx:
        const = ctx.enter_context(tc.tile_pool(name="const", bufs=1))

        ident = const.tile([P, P], FP32, tag="ident", name="ident")
        make_identity(nc, ident)
        ones16 = const.tile([P, P], BF16, tag="ones16", name="ones16")
        nc.vector.memset(ones16, 1.0)

        # proj, pair-packed [128, 256]: head 2i at partitions 0:64, head
        # 2i+1 at 64:128 (so lhsT/rhs partition bases always match).
        proj_all = const.tile([P, 4, F], BF16, tag="projp", name="projp")
        proj_pair = [proj_all[:, i, :] for i in range(4)]
        # w_out, needed only in pass B but loaded early while DMA is free
        wo_all = const.tile([P, 4, D], BF16, tag="wo", name="wo")
        wo_sb = [wo_all[:, s, :] for s in range(4)]

        # attn lhsT, zero-padded to M=128 so a head pair accumulates into one
        # [128,512] PSUM tile (matmul outputs must start at partition 0).
        kvS = [[const.tile([P, P], BF16, tag=f"kvS{h}_{s}", name=f"kvS{h}_{s}")
                for s in range(2)] for h in range(HPC)]
        # denominator lhsT: ksr[h][s] cols (h%2)*64.. replicate k_sum_h,
        # rest zero -> pair denominators land on the matching partitions of
        # one PSUM tile (z broadcast for free)
        ksr = [[const.tile([P, P], BF16, tag=f"ksr{h}_{s}", name=f"ksr{h}_{s}")
                for s in range(2)] for h in range(HPC)]
        for h in range(HPC):
            ho = HD - (h % 2) * HD
            for s in range(2):
                nc.vector.memset(kvS[h][s][:, ho:ho + HD], 0.0)
                nc.vector.memset(ksr[h][s][:, ho:ho + HD], 0.0)

        # qT resident in SBUF across both passes: 4 dh-slabs x [128, n] bf16
        qt_sb = [const.tile([P, n], BF16, tag=f"qt{s}", name=f"qt{s}")
                 for s in range(4)]
        # qP double-ring: [ring][head] -> [128, 1024] bf16 (feat x (s, tok)).
        # ring g%2 holds group g's projections; ring 0 of group 0 is warm-
        # started in the pass-A epilogue.
        qP_ring = [[const.tile([P, 1024], BF16, tag=f"qP{r}_{h}",
                               name=f"qP{r}_{h}") for h in range(HPC)]
                   for r in range(2)]

        # ---------------- pass A (group-pipelined) ----------------
        with ExitStack() as actx:
            # kv state: 4 persistent PSUM banks, accumulated over all groups
            kvpp = actx.enter_context(tc.tile_pool(name="kvpp", bufs=1,
                                                   space="PSUM"))
            kv_ps = [kvpp.tile([HD + 1, 512], FP32, tag=f"kv{i}", name=f"kv{i}")
                     for i in range(4)]
            kvtp = actx.enter_context(tc.tile_pool(name="kvtp", bufs=1))
            kvt_sb = [kvtp.tile([HD + 1, 512], FP32, tag=f"kvt{i}",
                                name=f"kvt{i}") for i in range(4)]

            wpool = actx.enter_context(tc.tile_pool(name="wpool", bufs=1))
            wq_all = wpool.tile([P, 8, DH], FP8, tag="wq", name="wq")
            wk_all = wpool.tile([P, 8, DH], FP8, tag="wk", name="wk")
            wv_all = wpool.tile([P, 8, DH], BF16, tag="wv", name="wv")
            wv_sb = [wv_all[:, s, :] for s in range(8)]

            xtpool = actx.enter_context(tc.tile_pool(name="xtpool", bufs=2))
            xt8pool = actx.enter_context(tc.tile_pool(name="xt8pool", bufs=2))
            ktpool = actx.enter_context(tc.tile_pool(name="ktpool", bufs=2))
            vpool = actx.enter_context(tc.tile_pool(name="vpool", bufs=2))
            elupool = actx.enter_context(tc.tile_pool(name="elupool", bufs=4))
            kppool = actx.enter_context(tc.tile_pool(name="kppool", bufs=20))
            # one shared 4-bank PSUM ring for qT/kT/v evict chase, kproj
            # quads, fixup transposes and the warm qproj (kv holds the
            # other 4 banks)
            mmps = actx.enter_context(tc.tile_pool(name="mmps", bufs=4, space="PSUM"))

            xT_v = xT.rearrange("(s p) m -> p s m", p=P)
            xT8_v = xT8.rearrange("(s p) m -> p s m", p=P)

            # startup DMAs: first qT needs wq8+xt8(0) (0.5MB each, parallel
            # queues); kT next (wk8); v needs xt(0)+wv; kproj needs proj.
            xt8_0 = xt8pool.tile([P, 8, 512], FP8, tag="xt8", name="xt8")
            nc.scalar.dma_start(out=xt8_0, in_=xT8_v[:, :, 0:512])
            nc.sync.dma_start(out=wq_all, in_=wq8.rearrange("(s p) m -> p s m", p=P))
            nc.sync.dma_start(out=wk_all, in_=wk8.rearrange("(s p) m -> p s m", p=P))
            xt0 = xtpool.tile([P, 8, 512], BF16, tag="xt", name="xt")
            nc.sync.dma_start(out=xt0, in_=xT_v[:, :, 0:512])
            nc.scalar.dma_start(out=proj_all, in_=proj.rearrange("(i p) f -> p i f", p=P))
            nc.scalar.dma_start(out=wv_all, in_=wv.rearrange("(s p) m -> p s m", p=P))
            nc.sync.dma_start(out=wo_all, in_=wout.rearrange("(s p) m -> p s m", p=P))

            # per-group state carried one iteration (group g processed for
            # kv in iteration g+1)
            kt_all = [None] * NG
            vone_all = [None] * NG
            kP_all = [[None] * 16 for _ in range(NG)]   # (hp, tp, h%2) -> idx

            def kv_quad(g, hp):
                """kv accumulation for (group g, head pair hp): 8 matmuls
                into the persistent PSUM bank kv_ps[hp]."""
                vone = vone_all[g]
                nmm = 0
                for tp in range(2):
                    for hh in range(2):
                        kP = kP_all[g][hp * 4 + tp * 2 + hh]
                        for ti in range(2):
                            t = tp * 2 + ti
                            nc.tensor.matmul(
                                kv_ps[hp][:, hh * F:(hh + 1) * F],
                                lhsT=(vone[:, t, 2 * hp + hh, :]),
                                rhs=(kP[:, ti * F:(ti + 1) * F]),
                                start=(g == 0 and nmm == 0),
                                stop=(g == NG - 1 and nmm == 7),
                                skip_group_check=True)
                            nmm += 1

            def warm_qproj(h):
                """group 0 qproj for head h into qP_ring[0][h] (pass-A
                epilogue; kpps psum + 512-wide elus)."""
                hp, hh = h // 2, h % 2
                hb = hh * HD
                for s in range(2):
                    cw = mmps.tile([P, 512], FP32, tag="mm", name="mm")
                    nc.tensor.matmul(
                        cw,
                        lhsT=(proj_pair[hp][hb:hb + HD, s * P:(s + 1) * P]),
                        rhs=(qt_sb[hp][hb:hb + HD, 0:512]),
                        start=True, stop=True)
                    emit_elu(cw, qP_ring[0][h][:, s * 512:(s + 1) * 512],
                             elupool, SECOND_A[(2 * h + s) % 16], "k")

            for g in range(NG + 1):
                if g < NG:
                    g0 = g * 512
                    if g == 0:
                        xt, xt8 = xt0, xt8_0
                    else:
                        xt8 = xt8pool.tile([P, 8, 512], FP8, tag="xt8", name="xt8")
                        nc.scalar.dma_start(out=xt8, in_=xT8_v[:, :, g0:g0 + 512])
                        xt = xtpool.tile([P, 8, 512], BF16, tag="xt", name="xt")
                        nc.sync.dma_start(out=xt, in_=xT_v[:, :, g0:g0 + 512])

                    # qT: fp8 DoubleRow (4 slab-pairs), de-scale at evict
                    for fs in range(4):
                        ps = mmps.tile([P, 512], FP32, tag="mm", name="mm")
                        for sp in range(4):
                            nc.tensor.matmul(
                                ps,
                                lhsT=(wq_all[:, 2 * sp:2 * sp + 2,
                                             fs * P:(fs + 1) * P]),
                                rhs=(xt8[:, 2 * sp:2 * sp + 2, :]),
                                start=(sp == 0), stop=(sp == 3),
                                perf_mode=DR)
                        nc.scalar.activation(qt_sb[fs][:, g0:g0 + 512], ps,
                                             AF.Copy, scale=1.0 / W8SCALE)

                    # kT: same (evict: scalar)
                    kt_sb = [ktpool.tile([P, 512], BF16, tag=f"kt{fs}", name=f"kt{fs}")
                             for fs in range(4)]
                    kt_all[g] = kt_sb
                    for fs in range(4):
                        ps = mmps.tile([P, 512], FP32, tag="mm", name="mm")
                        for sp in range(4):
                            nc.tensor.matmul(
                                ps,
                                lhsT=(wk_all[:, 2 * sp:2 * sp + 2,
                                             fs * P:(fs + 1) * P]),
                                rhs=(xt8[:, 2 * sp:2 * sp + 2, :]),
                                start=(sp == 0), stop=(sp == 3),
                                perf_mode=DR)
                        nc.scalar.activation(kt_sb[fs], ps,
                                             AF.Copy, scale=1.0 / W8SCALE)

                    # v with ones column (evict: vector), interleaved with
                    # kproj quads (this group) and kv quads (previous group)
                    vone = vpool.tile([P, TPG, HPC, HD + 1], BF16, tag="vone", name="vone")
                    vone_all[g] = vone
                    nc.vector.tensor_copy(
                        vone[:, :, :, HD],
                        ones16[:, 0:TPG * HPC].rearrange("p (t h) -> p t h", t=TPG))

                for j in range(TPG):
                    if g < NG:
                        t = j
                        ps = mmps.tile([P, 512], FP32, tag="mm", name="mm")
                        for s in range(8):
                            nc.tensor.matmul(
                                ps, lhsT=(xt[:, s, t * P:(t + 1) * P]),
                                rhs=(wv_sb[s]), start=(s == 0), stop=(s == 7))
                        nc.vector.tensor_copy(
                            vone_all[g][:, t, :, 0:HD],
                            ps.rearrange("p (h e) -> p h e", h=HPC))

                        # kproj quads for head pair hp=j
                        hp = j
                        for tp in range(2):
                            cps = {hh: mmps.tile([P, 512], FP32, tag="mm",
                                                 name="mm")
                                   for hh in range(2)}
                            for ti in range(2):
                                t2 = tp * 2 + ti
                                for hh in range(2):
                                    hb = hh * HD
                                    nc.tensor.matmul(
                                        cps[hh][:, ti * F:(ti + 1) * F],
                                        lhsT=(kt_all[g][hp][hb:hb + HD,
                                                            t2 * P:(t2 + 1) * P]),
                                        rhs=(proj_pair[hp][hb:hb + HD, :]),
                                        start=True, stop=True)
                            for hh in range(2):
                                ei = (hp * 4 + tp * 2 + hh) % 16
                                kP = kppool.tile([P, 512], BF16, tag="kP", name="kP")
                                emit_elu(cps[hh], kP, elupool,
                                         SECOND_A[ei], "k")
                                kP_all[g][hp * 4 + tp * 2 + hh] = kP

                    if g >= 1:
                        kv_quad(g - 1, j)
                        if g == NG:
                            # kv fixup for head pair j: PSUM -> SBUF, then
                            # transpose -> pair-packed kvS + replicated ksr
                            nc.vector.tensor_copy(kvt_sb[j], kv_ps[j])
                            for jj in range(2):   # head h = 2j + jj
                                h = 2 * j + jj
                                hb = jj * HD
                                for s in range(2):   # F slab
                                    tp = mmps.tile([P, 512], FP32,
                                                   tag="mm", name="mm")
                                    nc.tensor.transpose(
                                        tp[:, 0:HD + 1],
                                        kvt_sb[j][:, jj * F + s * P:
                                                  jj * F + (s + 1) * P],
                                        ident[0:HD + 1, 0:HD + 1])
                                    nc.vector.tensor_copy(
                                        kvS[h][s][:, hb:hb + HD],
                                        tp[:, 0:HD])
                                    nc.scalar.copy(
                                        ksr[h][s][:, hb:hb + HD],
                                        tp[:, HD:HD + 1].broadcast_to([P, HD]))
                            # warm-start group 0 qproj (2 heads per j)
                            warm_qproj(2 * j)
                            warm_qproj(2 * j + 1)

        # ---------------- pass B (gapless PE schedule) ----------------
        with ExitStack() as bctx:
            attpool = bctx.enter_context(tc.tile_pool(name="attpool", bufs=8))
            zpool = bctx.enter_context(tc.tile_pool(name="zpool", bufs=2))
            ypool = bctx.enter_context(tc.tile_pool(name="ypool", bufs=3))
            elupB = bctx.enter_context(tc.tile_pool(name="elupB", bufs=4))
            atps = bctx.enter_context(tc.tile_pool(name="atps", bufs=2, space="PSUM"))
            qpps = bctx.enter_context(tc.tile_pool(name="qpps", bufs=1, space="PSUM"))
            yps = bctx.enter_context(tc.tile_pool(name="yps", bufs=2, space="PSUM"))

            # att ring: [ring][hp] -> [128,512] bf16 (hd-pair x tok)
            att_ring = [[attpool.tile([P, 512], BF16, tag=f"att{r}_{i}",
                                      name=f"att{r}_{i}") for i in range(4)]
                        for r in range(2)]

            def qproj_head(g1, h):
                """qproj for head h of group g1 -> qP_ring[g1%2][h]."""
                hp, hh = h // 2, h % 2
                hb = hh * HD
                pss = qpps.tile([P, 1024], FP32, tag="qp", name="qp")
                for s in range(2):
                    nc.tensor.matmul(
                        pss[:, s * 512:(s + 1) * 512],
                        lhsT=(proj_pair[hp][hb:hb + HD, s * P:(s + 1) * P]),
                        rhs=(qt_sb[hp][hb:hb + HD, g1 * 512:g1 * 512 + 512]),
                        start=True, stop=True,
                        skip_group_check=True)
                emit_elu(pss, qP_ring[g1 % 2][h], elupB,
                         SECOND_B[h % 8], "q", w=1024)

            def attn_pair(g, hp):
                """attn+denom for head pair hp of group g; z; att_sb."""
                atp = atps.tile([P, 1024], FP32, tag="at", name="at")
                aps, dps = atp[:, 0:512], atp[:, 512:1024]
                nmm = 0
                for s in range(2):
                    for hh in range(2):
                        h = 2 * hp + hh
                        qP = qP_ring[g % 2][h]
                        nc.tensor.matmul(
                            aps, lhsT=(kvS[h][s]),
                            rhs=(qP[:, s * 512:(s + 1) * 512]),
                            start=(nmm == 0), stop=(nmm == 3),
                            skip_group_check=True)
                        nc.tensor.matmul(
                            dps, lhsT=(ksr[h][s]),
                            rhs=(qP[:, s * 512:(s + 1) * 512]),
                            start=(nmm == 0), stop=(nmm == 3),
                            skip_group_check=True)
                        nmm += 1
                zb = zpool.tile([P, 512], FP32, tag="zb", name="zb")
                nc.vector.reciprocal_approx_fast(zb, dps)
                nc.vector.tensor_tensor(
                    out=att_ring[g % 2][hp], in0=aps, in1=zb, op=AL.mult)

            y_sb_of_t = {}

            def y_chunk(g, t, o):
                """y[:, o-half] for token tile t of group g: 4 acc matmuls
                + evict (ACT for o=0, DVE for o=1) + DMA per t."""
                pso = yps.tile([P, 512], FP32, tag="yy", name="yy")
                for s in range(4):
                    nc.tensor.matmul(
                        pso,
                        lhsT=(att_ring[g % 2][s][:, t * P:(t + 1) * P]),
                        rhs=(wo_sb[s][:, o * 512:(o + 1) * 512]),
                        start=(s == 0), stop=(s == 3),
                        skip_group_check=True)
                if o == 0:
                    y_sb = ypool.tile([P, 1024], BF16, tag="ysb", name="ysb")
                    y_sb_of_t[t] = y_sb
                    nc.scalar.copy(y_sb[:, 0:512], pso)
                else:
                    y_sb = y_sb_of_t[t]
                    nc.vector.tensor_copy(y_sb[:, 512:1024], pso)
                    g0 = g * 512
                    nc.sync.dma_start(
                        out=y[g0 + t * P: g0 + (t + 1) * P, :], in_=y_sb)

            for g in range(NG):
                # A-phase: attn pairs + qproj(g+1) heads 0..3
                for hp in range(4):
                    attn_pair(g, hp)
                    if g + 1 < NG:
                        qproj_head(g + 1, hp)
                # Y-phase: y(g-1) chunks + qproj(g+1) heads 4..7
                if g > 0:
                    qi = 4
                    for i, (t, o) in enumerate(
                            [(t, o) for t in range(TPG) for o in range(2)]):
                        y_chunk(g - 1, t, o)
                        if i % 2 == 1 and g + 1 < NG and qi < 8:
                            qproj_head(g + 1, qi)
                            qi += 1
                elif g + 1 < NG:
                    for h in range(4, 8):
                        qproj_head(g + 1, h)
            for t in range(TPG):
                for o in range(2):
                    y_chunk(NG - 1, t, o)


def build(n=SEQ):
    nc = bacc.Bacc("TRN2", target_bir_lowering=False, debug=False,
                   enable_asserts=False)
    xT = nc.declare_dram_parameter("xT", [D, n], BF16, isOutput=False)
    xT8 = nc.declare_dram_parameter("xT8", [D, n], FP8, isOutput=False)
    wq8 = nc.declare_dram_parameter("wq8", [D, DH], FP8, isOutput=False)
    wk8 = nc.declare_dram_parameter("wk8", [D, DH], FP8, isOutput=False)
    wv = nc.declare_dram_parameter("wv", [D, DH], BF16, isOutput=False)
    proj = nc.declare_dram_parameter("proj", [DH, F], BF16, isOutput=False)
    wout = nc.declare_dram_parameter("wout", [DH, D], BF16, isOutput=False)
    y = nc.declare_dram_parameter("y", [n, D], BF16, isOutput=True)
    with tile.TileContext(nc) as tc:
        _emit(tc, n, xT, xT8, wq8, wk8, wv, proj, wout, y)
    nc.finalize()
    return nc


def make_in_maps(x, w_qkv, proj_matrix, w_out):
    bf = ml_dtypes.bfloat16
    f8 = ml_dtypes.float8_e4m3   # TRN FP8_EXP4 (max +-240)
    x = np.asarray(x, np.float32)
    w_qkv = np.asarray(w_qkv, np.float32)
    proj_matrix = np.asarray(proj_matrix, np.float32).astype(bf)
    w_out = np.asarray(w_out, np.float32).astype(bf)
    in_maps = []
    for c in range(NCORES):
        b, g = c // 2, c % 2
        xTb = np.ascontiguousarray(x[b].T)
        in_maps.append({
            "xT": xTb.astype(bf),
            "xT8": xTb.astype(f8),
            "wq8": np.ascontiguousarray(
                w_qkv[:, DH * g:DH * (g + 1)] * W8SCALE).astype(f8),
            "wk8": np.ascontiguousarray(
                w_qkv[:, D + DH * g:D + DH * (g + 1)] * W8SCALE).astype(f8),
            "wv": np.ascontiguousarray(
                w_qkv[:, 2 * D + DH * g:2 * D + DH * (g + 1)]).astype(bf),
            "proj": np.ascontiguousarray(
                proj_matrix[HPC * g:HPC * (g + 1)].reshape(DH, F)),
            "wout": np.ascontiguousarray(w_out[DH * g:DH * (g + 1), :]),
        })
    return in_maps


_NC_CACHE = {}


def get_nc(n=SEQ):
    if n not in _NC_CACHE:
        _NC_CACHE[n] = build(n)
    return _NC_CACHE[n]


def _install_ntff_hook_shim():
    """The agent image's antenv lacks axon_hooks; recreate it so
    run_bass_kernel_spmd(trace=True) can capture NTFF profiles."""
    import sys
    import types
    try:
        from antenv.axon_hooks import get_axon_ntff_profile_hook  # noqa: F401
        return True
    except ImportError:
        pass
    try:
        from trn_agent_boot.trn_boot import _ntff_profile_via_ctypes
        import antenv
        mod = types.ModuleType("antenv.axon_hooks")
        mod._hook = _ntff_profile_via_ctypes("/opt/axon/libaxon_pjrt.so")
        mod.set_axon_ntff_profile_hook = lambda h: setattr(mod, "_hook", h)
        mod.get_axon_ntff_profile_hook = lambda: mod._hook
        sys.modules["antenv.axon_hooks"] = mod
        antenv.axon_hooks = mod
        return True
    except Exception as e:  # profiling is best-effort
        print(f"ntff hook shim failed: {e}")
        return False


def run(x, w_qkv, proj_matrix, w_out, b_out, trace=False, **kw):
    if trace:
        _install_ntff_hook_shim()
    nc = get_nc(SEQ)
    in_maps = make_in_maps(x, w_qkv, proj_matrix, w_out)
    res = run_bass_kernel_spmd(nc, in_maps, list(range(NCORES)),
                               trace=trace, **kw)
    b_out = np.asarray(b_out, np.float32)
    out = np.empty((B, SEQ, D), np.float32)
    for b in range(B):
        out[b] = res.results[2 * b]["y"].astype(np.float32) \
            + res.results[2 * b + 1]["y"].astype(np.float32) \
            + b_out[None, :]
    return out, res


def kernel(x, w_qkv, proj_matrix, w_out, b_out):
    out, _ = run(x, w_qkv, proj_matrix, w_out, b_out)
    return out


# revision 8
# speedup vs baseline: 1.3078x; 1.3078x over previous
"""Trainium2 Bass kernel: Performer (linear) attention + in/out projections.

Problem nn_LinearPerformerAttention_6717328851263:
  x:(4,4096,1024) f32, w_qkv:(1024,3072), proj_matrix:(16,64,256),
  w_out:(1024,1024), b_out:(1024,)

  qkv = x @ w_qkv ; split q,k,v ; per (b,h): q_proj=elu1(q@P_h), k_proj=elu1(k@P_h)
  kv = k_proj^T v ; k_sum = sum_n k_proj ; attn = (q_proj @ kv) / (q_proj@k_sum)
  out = attn @ w_out + b_out

Sharding over 8 cores: core c -> (batch b=c//2, head-group g=c%2: 8 of 16 heads).
Each core computes partial y_c = attn(b, heads_g) @ w_out[512g:512g+512, :].
Host gather: out[b] = y_(b,0) + y_(b,1) + b_out.

v5: the kernel is ACT+DVE bound (~680us of scalar+vector work in v3 across
two engines), so the wins are op-count reduction and fp8:
  * qT/kT via fp8(e4m3) DoubleRow matmuls: same per-instruction wall time
    as bf16 but each contracts 2 k-slabs -> half the instructions/stream
    time.  x, wq, wk shipped in fp8 (weights pre-scaled x32 on the host,
    de-scaled at the PSUM evict).  Host-simmed rel err 7.9e-3 (gate 2e-2),
    measured 6.7e-3 on hw.
  * every ACT/DVE op widened to 1024 columns (fixed per-op overhead is
    ~300-900ns vs ~1ns/col marginal): qT/kT/v psums are [128,1024] pair
    tiles evicted in one op, kproj psum pairs both heads -> 8 wide elu
    tiles/group instead of 16 narrow ones.
  * pass A uses ONE shared [128,1024] PSUM ring (bufs=4 = all 8 banks);
    kv quads write [0:65, 0:512] slices of ring tiles, folded to SBUF as
    in v3.
  * pass B keeps v3's proven schedule (attn/denom -> z -> qproj(hp+2)
    interleave -> y stream) which sustained full PE clock; only the elu
    second-op engine patterns are retuned.
elu1(x) = elu(x)+1 = min(exp(x),1) + relu(x), emitted in two variants:
S-heavy (Exp+Relu on ACT, one stt on DVE) and V-heavy (Exp on ACT,
relu+1 and min on DVE), ratio tunable per pass.
"""

import numpy as np
from contextlib import ExitStack

import ml_dtypes
import concourse.bass as bass
import concourse.bacc as bacc
import concourse.tile as tile
from concourse import mybir
from concourse.bass_utils import run_bass_kernel_spmd
from concourse.masks import make_identity

FP32 = mybir.dt.float32
BF16 = mybir.dt.bfloat16
FP8 = mybir.dt.float8e4
AL = mybir.AluOpType
AF = mybir.ActivationFunctionType
DR = mybir.MatmulPerfMode.DoubleRow

B, SEQ, D = 4, 4096, 1024
H, HD, F = 16, 64, 256
HPC = 8            # heads per core
DH = HPC * HD      # 512 head-space dims per core
P = 128
NCORES = 8
W8SCALE = 32.0     # host pre-scale on fp8 wq/wk (w std 1/32 -> ~1)

# elu1 second-op engine patterns: 'S' = ACT Relu, 'V' = DVE max(c,0)+1.
# (third op always DVE; GPSIMD rejects min, DMA/Pool can't touch PSUM)
SECOND_A = "VVVSVVVS"   # pass A: 8 wide kproj tiles/group
SECOND_B = "SVSSVSSV"   # pass B: 8 qproj head-tiles/group (hp*2+hh)


def _emit(tc, n, xT, xT8, wq8, wk8, wv, proj, wout, y):
    nc = tc.nc
    NG = n // 512       # token groups
    TPG = 4             # 128-token tiles per group

    def emit_elu(c_ps, out, pool, second, tag, w=1024):
        """elu1 = min(exp(c),1) + relu(c), c in PSUM fp32 -> bf16 `out`."""
        e = pool.tile([P, w], BF16, tag=f"{tag}E", name=f"{tag}E")
        r = pool.tile([P, w], BF16, tag=f"{tag}R", name=f"{tag}R")
        nc.scalar.activation(e, c_ps, AF.Exp)
        if second == "S":
            nc.scalar.activation(r, c_ps, AF.Relu)
            nc.vector.scalar_tensor_tensor(
                out, in0=e, scalar=1.0, in1=r, op0=AL.min, op1=AL.add)
        else:
            # r = relu(c)+1 ; out = min(e, r)  (== min(e,1)+relu(c))
            nc.vector.tensor_scalar(r, c_ps, 0.0, 1.0, op0=AL.max, op1=AL.add)
            nc.vector.tensor_tensor(out=out, in0=e, in1=r, op=AL.min)

    ctx = ExitStack()
    with ctx:
        const = ctx.enter_context(tc.tile_pool(name="const", bufs=1))

        ident = const.tile([P, P], FP32, tag="ident", name="ident")
        make_identity(nc, ident)
        ones16 = const.tile([P, P], BF16, tag="ones16", name="ones16")
        nc.vector.memset(ones16, 1.0)

        # proj, pair-packed [128, 256]: head 2i at partitions 0:64, head
        # 2i+1 at 64:128 (so lhsT/rhs partition bases always match).
        proj_all = const.tile([P, 4, F], BF16, tag="projp", name="projp")
        proj_pair = [proj_all[:, i, :] for i in range(4)]
        wo_all = const.tile([P, 4, D], BF16, tag="wo", name="wo")
        wo_sb = [wo_all[:, s, :] for s in range(4)]

        # attn lhsT, zero-padded to M=128 so a head pair accumulates into one
        # [128,512] PSUM tile; ksr replicates k_sum over the head's 64 cols.
        kvS = [[const.tile([P, P], BF16, tag=f"kvS{h}_{s}", name=f"kvS{h}_{s}")
                for s in range(2)] for h in range(HPC)]
        ksr = [[const.tile([P, P], BF16, tag=f"ksr{h}_{s}", name=f"ksr{h}_{s}")
                for s in range(2)] for h in range(HPC)]
        for h in range(HPC):
            ho = HD - (h % 2) * HD
            for s in range(2):
                nc.vector.memset(kvS[h][s][:, ho:ho + HD], 0.0)
                nc.vector.memset(ksr[h][s][:, ho:ho + HD], 0.0)

        # qT resident in SBUF across both passes: [128, fs, n] bf16 (single
        # tile so fs-pair evicts can be one wide ACT op)
        qt_all = const.tile([P, 4, n], BF16, tag="qt", name="qt")
        qt_sb = [qt_all[:, s, :] for s in range(4)]
        # warm-start tiles: group 0's qP for pairs 0/1, computed during the
        # pass-A epilogue so pass B's first attn doesn't wait on its prologue
        qP_warm = [const.tile([P, 1024], BF16, tag=f"qPw{i}", name=f"qPw{i}")
                   for i in range(4)]

        # ---------------- pass A (group-pipelined) ----------------
        with ExitStack() as actx:
            kvaccp = actx.enter_context(tc.tile_pool(name="kvaccp", bufs=1))
            kv_acc = [kvaccp.tile([HD + 1, 512], FP32, tag=f"kva{i}", name=f"kva{i}")
                      for i in range(4)]
            wpool = actx.enter_context(tc.tile_pool(name="wpool", bufs=1))
            wq_all = wpool.tile([P, 8, DH], FP8, tag="wq", name="wq")
            wk_all = wpool.tile([P, 8, DH], FP8, tag="wk", name="wk")
            wv_all = wpool.tile([P, 8, DH], BF16, tag="wv", name="wv")
            wv_sb = [wv_all[:, s, :] for s in range(8)]

            xtpool = actx.enter_context(tc.tile_pool(name="xtpool", bufs=2))
            xt8pool = actx.enter_context(tc.tile_pool(name="xt8pool", bufs=2))
            ktpool = actx.enter_context(tc.tile_pool(name="ktpool", bufs=2))
            vpool = actx.enter_context(tc.tile_pool(name="vpool", bufs=2))
            elupool = actx.enter_context(tc.tile_pool(name="elupool", bufs=4))
            kppool = actx.enter_context(tc.tile_pool(name="kppool", bufs=10))
            # ONE shared PSUM ring: [128,1024] x 4 bufs = all 8 banks.
            # qT/kT/v/kproj pair tiles, kv-quad [0:65,0:512] slices,
            # fixup transposes and the warm qproj all allocate from it.
            mmps = actx.enter_context(tc.tile_pool(name="mmps", bufs=4, space="PSUM"))

            xT_v = xT.rearrange("(s p) m -> p s m", p=P)
            xT8_v = xT8.rearrange("(s p) m -> p s m", p=P)

            # startup: first qT needs wq8 + xt8(0) (0.5MB each, parallel
            # queues); then wk8 (kT), xt+wv (v), proj (kproj).
            xt8_0 = xt8pool.tile([P, 8, 512], FP8, tag="xt8", name="xt8")
            nc.scalar.dma_start(out=xt8_0, in_=xT8_v[:, :, 0:512])
            nc.sync.dma_start(out=wq_all, in_=wq8.rearrange("(s p) m -> p s m", p=P))
            nc.sync.dma_start(out=wk_all, in_=wk8.rearrange("(s p) m -> p s m", p=P))
            xt0 = xtpool.tile([P, 8, 512], BF16, tag="xt", name="xt")
            nc.sync.dma_start(out=xt0, in_=xT_v[:, :, 0:512])
            nc.scalar.dma_start(out=proj_all, in_=proj.rearrange("(i p) f -> p i f", p=P))
            nc.scalar.dma_start(out=wv_all, in_=wv.rearrange("(s p) m -> p s m", p=P))
            nc.sync.dma_start(out=wo_all, in_=wout.rearrange("(s p) m -> p s m", p=P))

            kt_all = [None] * NG
            vone_all = [None] * NG
            kP_all = [[None] * 8 for _ in range(NG)]   # (hp, tp) -> pair tile

            def kv_quad(g, hp):
                """kv accumulation for (group g, head pair hp): 8 matmuls
                into a [0:65, 0:512] slice of a shared-ring tile + fold."""
                vone = vone_all[g]
                kvt = mmps.tile([P, 1024], FP32, tag="mm", name="mm")
                kv_ps = kvt[0:HD + 1, 0:512]
                nmm = 0
                for tp in range(2):
                    for hh in range(2):
                        kP = kP_all[g][hp * 2 + tp]
                        for ti in range(2):
                            nc.tensor.matmul(
                                kv_ps[:, hh * F:(hh + 1) * F],
                                lhsT=(vone[:, tp * 2 + ti, 2 * hp + hh, :]),
                                rhs=(kP[:, hh * 512 + ti * F:
                                        hh * 512 + (ti + 1) * F]),
                                start=(nmm == 0), stop=(nmm == 7),
                                skip_group_check=True)
                            nmm += 1
                if g == 0:
                    nc.vector.tensor_copy(kv_acc[hp], kv_ps)
                else:
                    nc.vector.tensor_tensor(
                        out=kv_acc[hp], in0=kv_ps, in1=kv_acc[hp], op=AL.add)

            for g in range(NG + 1):
                if g < NG:
                    g0 = g * 512
                    if g == 0:
                        xt, xt8 = xt0, xt8_0
                    else:
                        xt8 = xt8pool.tile([P, 8, 512], FP8, tag="xt8", name="xt8")
                        nc.sync.dma_start(out=xt8, in_=xT8_v[:, :, g0:g0 + 512])
                        xt = xtpool.tile([P, 8, 512], BF16, tag="xt", name="xt")
                        nc.sync.dma_start(out=xt, in_=xT_v[:, :, g0:g0 + 512])

                    # qT: fp8 DoubleRow, fs-pairs share a [128,1024] psum,
                    # one wide evict (de-scale 1/32) into qt_all
                    for fp in range(2):
                        ps = mmps.tile([P, 1024], FP32, tag="mm", name="mm")
                        for fi in range(2):
                            fs = fp * 2 + fi
                            for sp in range(4):
                                nc.tensor.matmul(
                                    ps[:, fi * 512:(fi + 1) * 512],
                                    lhsT=(wq_all[:, 2 * sp:2 * sp + 2,
                                                 fs * P:(fs + 1) * P]),
                                    rhs=(xt8[:, 2 * sp:2 * sp + 2, :]),
                                    start=(sp == 0), stop=(sp == 3),
                                    perf_mode=DR, skip_group_check=True)
                        nc.scalar.activation(
                            qt_all[:, 2 * fp:2 * fp + 2, g0:g0 + 512], ps,
                            AF.Copy, scale=1.0 / W8SCALE)

                    # kT: same; per-group [128, 4, 512] tile
                    kt_g = ktpool.tile([P, 4, 512], BF16, tag="kt", name="kt")
                    kt_all[g] = kt_g
                    for fp in range(2):
                        ps = mmps.tile([P, 1024], FP32, tag="mm", name="mm")
                        for fi in range(2):
                            fs = fp * 2 + fi
                            for sp in range(4):
                                nc.tensor.matmul(
                                    ps[:, fi * 512:(fi + 1) * 512],
                                    lhsT=(wk_all[:, 2 * sp:2 * sp + 2,
                                                 fs * P:(fs + 1) * P]),
                                    rhs=(xt8[:, 2 * sp:2 * sp + 2, :]),
                                    start=(sp == 0), stop=(sp == 3),
                                    perf_mode=DR, skip_group_check=True)
                        nc.scalar.activation(
                            kt_g[:, 2 * fp:2 * fp + 2, :], ps,
                            AF.Copy, scale=1.0 / W8SCALE)

                    vone = vpool.tile([P, TPG, HPC, HD + 1], BF16, tag="vone", name="vone")
                    vone_all[g] = vone
                    nc.vector.tensor_copy(
                        vone[:, :, :, HD],
                        ones16[:, 0:TPG * HPC].rearrange("p (t h) -> p t h", t=TPG))

                for j in range(TPG):
                    if g < NG:
                        # v for a token-tile pair (j even): [128,1024] psum,
                        # one wide evict into vone
                        if j % 2 == 0:
                            ps = mmps.tile([P, 1024], FP32, tag="mm", name="mm")
                            for ti in range(2):
                                t = j + ti
                                for s in range(8):
                                    nc.tensor.matmul(
                                        ps[:, ti * 512:(ti + 1) * 512],
                                        lhsT=(xt[:, s, t * P:(t + 1) * P]),
                                        rhs=(wv_sb[s]), start=(s == 0),
                                        stop=(s == 7), skip_group_check=True)
                            nc.vector.tensor_copy(
                                vone_all[g][:, j:j + 2, :, 0:HD],
                                ps.rearrange("p (t h e) -> p t h e", t=2, h=HPC))

                        # kproj for head pair hp=j: per tp one [128,1024]
                        # psum pairing both heads (hh0 cols 0:512, hh1
                        # 512:1024; inner ti x F), one wide elu -> kP pair
                        hp = j
                        for tp in range(2):
                            cps = mmps.tile([P, 1024], FP32, tag="mm", name="mm")
                            for ti in range(2):
                                t2 = tp * 2 + ti
                                for hh in range(2):
                                    hb = hh * HD
                                    nc.tensor.matmul(
                                        cps[:, hh * 512 + ti * F:
                                            hh * 512 + (ti + 1) * F],
                                        lhsT=(kt_all[g][hb:hb + HD, hp,
                                                        t2 * P:(t2 + 1) * P]),
                                        rhs=(proj_pair[hp][hb:hb + HD, :]),
                                        start=True, stop=True,
                                        skip_group_check=True)
                            ei = (hp * 2 + tp) % 8
                            kP = kppool.tile([P, 1024], BF16, tag="kP", name="kP")
                            emit_elu(cps, kP, elupool, SECOND_A[ei], "k")
                            kP_all[g][hp * 2 + tp] = kP

                    if g >= 1:
                        kv_quad(g - 1, j)
                        if g == NG:
                            # kv fixup for head pair j (kvS/ksr packing),
                            # interleaved with the epilogue kv quads
                            kvt_sb = kv_acc[j]
                            for jj in range(2):   # head h = 2j + jj
                                h = 2 * j + jj
                                hb = jj * HD
                                for s in range(2):   # F slab
                                    tpt = mmps.tile([P, 1024], FP32,
                                                    tag="mm", name="mm")
                                    tp = tpt[:, 0:HD + 1]
                                    nc.tensor.transpose(
                                        tp,
                                        kvt_sb[:, jj * F + s * P:
                                               jj * F + (s + 1) * P],
                                        ident[0:HD + 1, 0:HD + 1])
                                    nc.vector.tensor_copy(
                                        kvS[h][s][:, hb:hb + HD], tp[:, 0:HD])
                                    nc.scalar.copy(
                                        ksr[h][s][:, hb:hb + HD],
                                        tp[:, HD:HD + 1].broadcast_to([P, HD]))
                            if j < 2:
                                # warm-start: group 0's qproj+elu for pair j
                                for hh in range(2):
                                    hb2 = hh * HD
                                    cw = mmps.tile([P, 1024], FP32,
                                                   tag="mm", name="mm")
                                    for s in range(2):
                                        nc.tensor.matmul(
                                            cw[:, s * 512:(s + 1) * 512],
                                            lhsT=(proj_pair[j][hb2:hb2 + HD,
                                                               s * P:(s + 1) * P]),
                                            rhs=(qt_sb[j][hb2:hb2 + HD, 0:512]),
                                            start=True, stop=True,
                                            skip_group_check=True)
                                    emit_elu(cw, qP_warm[2 * j + hh], elupool,
                                             SECOND_A[(j * 2 + hh) % 8], "k")

        # ---------------- pass B (pair-pipelined, v3 schedule) ----------------
        with ExitStack() as bctx:
            qppool = bctx.enter_context(tc.tile_pool(name="qppool", bufs=4))
            qPpool = bctx.enter_context(tc.tile_pool(name="qPpool", bufs=3))
            attpool = bctx.enter_context(tc.tile_pool(name="attpool", bufs=2))
            zpool = bctx.enter_context(tc.tile_pool(name="zpool", bufs=2))
            ypool = bctx.enter_context(tc.tile_pool(name="ypool", bufs=3))
            qpps = bctx.enter_context(tc.tile_pool(name="qpps", bufs=2, space="PSUM"))
            atps = bctx.enter_context(tc.tile_pool(name="atps", bufs=1, space="PSUM"))
            yps = bctx.enter_context(tc.tile_pool(name="yps", bufs=1, space="PSUM"))

            qP_next = [None, None]   # pairs 0/1 of group g, made in g-1
            for g in range(NG):
                g0 = g * 512

                qP_pairs = [None] * 4

                def qproj_pair(hp, goff, dest, di):
                    """qproj matmuls + elu for head pair hp at token offset
                    goff; both F-slabs of one head share a [128,1024]
                    2-bank PSUM tile -> 1024-wide elu."""
                    pss = {hh: qpps.tile([P, 1024], FP32, tag="qp", name="qp")
                           for hh in range(2)}
                    for s in range(2):
                        for hh in range(2):   # even/odd interleave (PE rows)
                            hb = hh * HD
                            nc.tensor.matmul(
                                pss[hh][:, s * 512:(s + 1) * 512],
                                lhsT=(proj_pair[hp][hb:hb + HD,
                                                    s * P:(s + 1) * P]),
                                rhs=(qt_sb[hp][hb:hb + HD, goff:goff + 512]),
                                start=True, stop=True,
                                skip_group_check=True)
                    qPs = []
                    for hh in range(2):
                        qP = qPpool.tile([P, 1024], BF16, tag=f"qP{hh}",
                                         name=f"qP{hh}")
                        emit_elu(pss[hh], qP, qppool,
                                 SECOND_B[(hp * 2 + hh) % 8], "q")
                        qPs.append((hh, qP))
                    dest[di] = qPs

                att_sb = [attpool.tile([P, 512], BF16, tag=f"att{i}", name=f"att{i}")
                          for i in range(4)]

                if g == 0:
                    qP_pairs[0] = [(0, qP_warm[0]), (1, qP_warm[1])]
                    qP_pairs[1] = [(0, qP_warm[2]), (1, qP_warm[3])]
                else:
                    qP_pairs[0] = qP_next[0]
                    qP_pairs[1] = qP_next[1]
                for hp in range(4):
                    aps = atps.tile([P, 512], FP32, tag="at", name="aps")
                    dps = atps.tile([P, 512], FP32, tag="dn", name="dn")
                    nmm = 0
                    for s in range(2):
                        for (hh, qP) in qP_pairs[hp]:
                            h = 2 * hp + hh
                            nc.tensor.matmul(
                                aps, lhsT=(kvS[h][s]),
                                rhs=(qP[:, s * 512:(s + 1) * 512]),
                                start=(nmm == 0), stop=(nmm == 3),
                                skip_group_check=True)
                            nc.tensor.matmul(
                                dps, lhsT=(ksr[h][s]),
                                rhs=(qP[:, s * 512:(s + 1) * 512]),
                                start=(nmm == 0), stop=(nmm == 3),
                                skip_group_check=True)
                            nmm += 1
                    zb = zpool.tile([P, 512], FP32, tag="zb", name="zb")
                    nc.vector.reciprocal_approx_fast(zb, dps)
                    nc.vector.tensor_tensor(
                        out=att_sb[hp], in0=aps, in1=zb, op=AL.mult)
                    if hp + 2 < 4:
                        qproj_pair(hp + 2, g0, qP_pairs, hp + 2)

                # next group's first two qproj pairs BEFORE the y stream -
                # their elu latency hides under the 32 y matmuls
                if g + 1 < NG:
                    qproj_pair(0, g0 + 512, qP_next, 0)
                    qproj_pair(1, g0 + 512, qP_next, 1)

                # y = attn^T @ w_out; both o-halves land in one 2-bank
                # PSUM tile -> a single evict + DMA per token tile
                for t in range(TPG):
                    pso = yps.tile([P, 1024], FP32, tag="yy", name="yy")
                    for s in range(4):
                        for o in range(2):
                            nc.tensor.matmul(
                                pso[:, o * 512:(o + 1) * 512],
                                lhsT=(att_sb[s][:, t * P:(t + 1) * P]),
                                rhs=(wo_sb[s][:, o * 512:(o + 1) * 512]),
                                start=(s == 0), stop=(s == 3),
                                skip_group_check=True)
                    y_sb = ypool.tile([P, 1024], BF16, tag="ysb", name="ysb")
                    if t % 2 == 0:
                        nc.scalar.copy(y_sb, pso)
                    else:
                        nc.vector.tensor_copy(y_sb, pso)
                    nc.sync.dma_start(
                        out=y[g0 + t * P: g0 + (t + 1) * P, :], in_=y_sb)


def build(n=SEQ):
    nc = bacc.Bacc("TRN2", target_bir_lowering=False, debug=False,
                   enable_asserts=False)
    xT = nc.declare_dram_parameter("xT", [D, n], BF16, isOutput=False)
    xT8 = nc.declare_dram_parameter("xT8", [D, n], FP8, isOutput=False)
    wq8 = nc.declare_dram_parameter("wq8", [D, DH], FP8, isOutput=False)
    wk8 = nc.declare_dram_parameter("wk8", [D, DH], FP8, isOutput=False)
    wv = nc.declare_dram_parameter("wv", [D, DH], BF16, isOutput=False)
    proj = nc.declare_dram_parameter("proj", [DH, F], BF16, isOutput=False)
    wout = nc.declare_dram_parameter("wout", [DH, D], BF16, isOutput=False)
    y = nc.declare_dram_parameter("y", [n, D], BF16, isOutput=True)
    with tile.TileContext(nc) as tc:
        _emit(tc, n, xT, xT8, wq8, wk8, wv, proj, wout, y)
    nc.finalize()
    return nc


def make_in_maps(x, w_qkv, proj_matrix, w_out):
    bf = ml_dtypes.bfloat16
    f8 = ml_dtypes.float8_e4m3   # TRN FP8_EXP4 (max +-240)
    x = np.asarray(x, np.float32)
    w_qkv = np.asarray(w_qkv, np.float32)
    proj_matrix = np.asarray(proj_matrix, np.float32).astype(bf)
    w_out = np.asarray(w_out, np.float32).astype(bf)
    in_maps = []
    for c in range(NCORES):
        b, g = c // 2, c % 2
        xTb = np.ascontiguousarray(x[b].T)
        in_maps.append({
            "xT": xTb.astype(bf),
            "xT8": xTb.astype(f8),
            "wq8": np.ascontiguousarray(
                w_qkv[:, DH * g:DH * (g + 1)] * W8SCALE).astype(f8),
            "wk8": np.ascontiguousarray(
                w_qkv[:, D + DH * g:D + DH * (g + 1)] * W8SCALE).astype(f8),
            "wv": np.ascontiguousarray(
                w_qkv[:, 2 * D + DH * g:2 * D + DH * (g + 1)]).astype(bf),
            "proj": np.ascontiguousarray(
                proj_matrix[HPC * g:HPC * (g + 1)].reshape(DH, F)),
            "wout": np.ascontiguousarray(w_out[DH * g:DH * (g + 1), :]),
        })
    return in_maps


_NC_CACHE = {}


def get_nc(n=SEQ):
    if n not in _NC_CACHE:
        _NC_CACHE[n] = build(n)
    return _NC_CACHE[n]


def _install_ntff_hook_shim():
    """The agent image's antenv lacks axon_hooks; recreate it so
    run_bass_kernel_spmd(trace=True) can capture NTFF profiles."""
    import sys
    import types
    try:
        from antenv.axon_hooks import get_axon_ntff_profile_hook  # noqa: F401
        return True
    except ImportError:
        pass
    try:
        from trn_agent_boot.trn_boot import _ntff_profile_via_ctypes
        import antenv
        mod = types.ModuleType("antenv.axon_hooks")
        mod._hook = _ntff_profile_via_ctypes("/opt/axon/libaxon_pjrt.so")
        mod.set_axon_ntff_profile_hook = lambda h: setattr(mod, "_hook", h)
        mod.get_axon_ntff_profile_hook = lambda: mod._hook
        sys.modules["antenv.axon_hooks"] = mod
        antenv.axon_hooks = mod
        return True
    except Exception as e:  # profiling is best-effort
        print(f"ntff hook shim failed: {e}")
        return False


def run(x, w_qkv, proj_matrix, w_out, b_out, trace=False, **kw):
    if trace:
        _install_ntff_hook_shim()
    nc = get_nc(SEQ)
    in_maps = make_in_maps(x, w_qkv, proj_matrix, w_out)
    res = run_bass_kernel_spmd(nc, in_maps, list(range(NCORES)),
                               trace=trace, **kw)
    b_out = np.asarray(b_out, np.float32)
    out = np.empty((B, SEQ, D), np.float32)
    for b in range(B):
        out[b] = res.results[2 * b]["y"].astype(np.float32) \
            + res.results[2 * b + 1]["y"].astype(np.float32) \
            + b_out[None, :]
    return out, res


def kernel(x, w_qkv, proj_matrix, w_out, b_out):
    out, _ = run(x, w_qkv, proj_matrix, w_out, b_out)
    return out
